# revision 1
# baseline (speedup 1.0000x reference)
"""Causal multi-head attention (B=4, T=2048, C=1024, H=16) on 8 TRN2 cores.

Sharding: batch (4) x head-group (2 groups of 8 heads) -> 8 shards, one per
core. Each core computes QKV projections for its 8 heads, causal flash-style
attention, and a Megatron row-parallel slice of the output projection; the
host sums the two head-group partial outputs per batch element.

Per-core dataflow (all matmuls in float32r, 1 PE cycle/row at N>=256):
  phase 1a: V   = xT c-tiles (lhsT) @ wvT -> [t,dv] -> resident V_aug tiles
  phase 1b: Q^T,K^T = wq/wkT (lhsT) @ xT  -> [f,t]; Q resident, K spilled
  phase 2:  per (head, 512-query block): S^T = K^T.T @ Q^T per 128-k tile
            (diagonal staircase blocks trimmed + packed into one 3-bank psum
            tile), P^T = exp(S^T/8) (ACT; mask multiply on DVE), PV^T
            accumulated with V_aug stationary -> [d+1, q] (row 64 = l),
            normalize via DVE recip + PE ones-broadcast -> ctx^T [c,t]
  phase 3:  y^T = woT (lhsT) @ ctx^T + bias -> [o,t] -> DRAM

Self-contained: hardcodes shapes from the problem spec; no file reads.
"""
import sys
sys.path.insert(0, '/opt/trn_rl_repo')
import numpy as np

B, T, C = 4, 2048, 1024
H, D = 16, 64
N_CORES = 8
HPC = 8        # heads per core
HP = 4         # head pairs per core
KB = 16        # 128-row key tiles per sequence
NQSB = 4       # 512-column query superblocks
CI = 8         # 128-row contraction tiles over C
VW = 66        # V_aug stride per head (64 V + 1 ones + 1 pad)

# Diagonal-staircase packing inside one [128, 1536] psum tile: block j covers
# query range [QOFF[j], 512) of the superblock, lives at psum column POFF[j].
QOFF = (0, 128, 256, 256)
POFF = (0, 512, 896, 1152)
MW = 1408      # merged mask width (gapless staircase packing)

_CACHE = {}


def build_nc(iters=1):
    import contextlib
    import concourse.tile as tile
    from concourse import bacc, mybir

    F32 = mybir.dt.float32
    F32R = mybir.dt.float32r
    EXP = mybir.ActivationFunctionType.Exp
    IDENT = mybir.ActivationFunctionType.Identity

    nc = bacc.Bacc("TRN2", target_bir_lowering=False, debug=False)

    xT_d = nc.dram_tensor("xT", [C, T], F32R, kind="ExternalInput")
    wqT_d = nc.dram_tensor("wqT", [C, 512], F32R, kind="ExternalInput")
    wkT_d = nc.dram_tensor("wkT", [C, 512], F32R, kind="ExternalInput")
    wvT_d = nc.dram_tensor("wvT", [C, 512], F32R, kind="ExternalInput")
    woT_d = nc.dram_tensor("woT", [512, C], F32R, kind="ExternalInput")
    bias_d = nc.dram_tensor("bias", [128, 8], F32, kind="ExternalInput")
    mask_d = nc.dram_tensor("masks", [128, MW], F32R, kind="ExternalInput")
    yT_d = nc.dram_tensor("yT", [C, T], F32, kind="ExternalOutput")
    kT_spill = nc.dram_tensor("kT_spill", [512, T], F32R)

    with tile.TileContext(nc) as tc:
        def emit():
            with contextlib.ExitStack() as es:
                const = es.enter_context(tc.tile_pool(name="const", bufs=1))
                qtp = es.enter_context(tc.tile_pool(name="qt", bufs=1))
                ctxp = es.enter_context(tc.tile_pool(name="ctx", bufs=1))
                vp = es.enter_context(tc.tile_pool(name="vsb", bufs=1))

                ones_f = const.tile([128, 64], F32)
                nc.any.memset(ones_f[:], 1.0)
                ones_r = const.tile([128, 64], F32R)
                nc.vector.tensor_copy(ones_r[:], ones_f[:])
                ones16_f = const.tile([128, 16], F32)
                nc.any.memset(ones16_f[:], 1.0)
                ones16_r = const.tile([128, 16], F32R)
                nc.vector.tensor_copy(ones16_r[:], ones16_f[:])
                bias_sb = const.tile([128, 8], F32)
                nc.sync.dma_start(bias_sb[:], bias_d.ap())

                qt_sb, ctx_sb, v_sb = [], [], []
                for hp in range(HP):
                    qt_sb.append(qtp.tile([128, T], F32R, tag=f"qt{hp}",
                                          name=f"qt{hp}"))
                    ctx_sb.append(ctxp.tile([128, T], F32R, tag=f"ctx{hp}",
                                            name=f"ctx{hp}"))
                for kb in range(KB):
                    v_sb.append(vp.tile([128, HPC * VW], F32R, tag=f"v{kb}",
                                        name=f"v{kb}"))

                # ---------------- phase 1: projections ----------------
                with contextlib.ExitStack() as p1:
                    xtp = p1.enter_context(tc.tile_pool(name="xt", bufs=1))
                    xt_sb = []
                    for ci in range(CI):
                        t_ = xtp.tile([128, T], F32R, tag=f"xt{ci}")
                        nc.sync.dma_start(t_[:],
                                          xT_d.ap()[ci * 128:(ci + 1) * 128, :])
                        xt_sb.append(t_)

                    # --- 1a: V (resident V_aug tiles) ---
                    with contextlib.ExitStack() as p1a:
                        wvp = p1a.enter_context(tc.tile_pool(name="wv", bufs=1))
                        vps = p1a.enter_context(
                            tc.tile_pool(name="vps", bufs=4, space="PSUM"))
                        wv_sb = []
                        for ci in range(CI):
                            t_ = wvp.tile([128, 512], F32R, tag=f"wv{ci}")
                            nc.sync.dma_start(
                                t_[:], wvT_d.ap()[ci * 128:(ci + 1) * 128, :])
                            wv_sb.append(t_)
                        for ti in range(KB):
                            ps_ = vps.tile([128, 512], F32)
                            for ci in range(CI):
                                nc.tensor.matmul(
                                    ps_[:],
                                    xt_sb[ci][:, ti * 128:(ti + 1) * 128],
                                    wv_sb[ci][:],
                                    start=(ci == 0), stop=(ci == CI - 1),
                                    skip_group_check=True)
                            sv = v_sb[ti][:].rearrange("p (h w) -> p h w", w=VW)
                            nc.vector.tensor_copy(
                                sv[:, :, 64:66],
                                ones16_r[:].rearrange("p (h w) -> p h w", w=2))
                            nc.vector.tensor_copy(
                                sv[:, :, 0:64],
                                ps_[:].rearrange("p (h w) -> p h w", w=64))

                    # --- 1b: Q^T, K^T (per head pair) ---
                    with contextlib.ExitStack() as p1b:
                        wqp = p1b.enter_context(tc.tile_pool(name="wq", bufs=2))
                        wkp = p1b.enter_context(tc.tile_pool(name="wk", bufs=2))
                        kstg = p1b.enter_context(tc.tile_pool(name="kstg", bufs=4))
                        qkps = p1b.enter_context(
                            tc.tile_pool(name="qkps", bufs=3, space="PSUM"))
                        for hp in range(HP):
                            fsl = slice(hp * 128, (hp + 1) * 128)
                            wq_sb, wk_sb = [], []
                            for ci in range(CI):
                                tq = wqp.tile([128, 128], F32R, tag=f"wqs{ci}")
                                nc.sync.dma_start(
                                    tq[:], wqT_d.ap()[ci * 128:(ci + 1) * 128, fsl])
                                wq_sb.append(tq)
                                tk = wkp.tile([128, 128], F32R, tag=f"wks{ci}")
                                nc.sync.dma_start(
                                    tk[:], wkT_d.ap()[ci * 128:(ci + 1) * 128, fsl])
                                wk_sb.append(tk)
                            for tj in range(NQSB):
                                tsl = slice(tj * 512, (tj + 1) * 512)
                                ps_ = qkps.tile([128, 512], F32)
                                for ci in range(CI):
                                    nc.tensor.matmul(
                                        ps_[:], wq_sb[ci][:], xt_sb[ci][:, tsl],
                                        start=(ci == 0), stop=(ci == CI - 1),
                                        skip_group_check=True)
                                nc.scalar.copy(qt_sb[hp][:, tsl], ps_[:])
                                ps2 = qkps.tile([128, 512], F32, tag="psk")
                                for ci in range(CI):
                                    nc.tensor.matmul(
                                        ps2[:], wk_sb[ci][:], xt_sb[ci][:, tsl],
                                        start=(ci == 0), stop=(ci == CI - 1),
                                        skip_group_check=True)
                                stg = kstg.tile([128, 512], F32R)
                                nc.vector.tensor_copy(stg[:], ps2[:])
                                nc.sync.dma_start(kT_spill.ap()[fsl, tsl], stg[:])

                # ---------------- phase 2: attention ----------------
                with contextlib.ExitStack() as p2:
                    maskp = p2.enter_context(tc.tile_pool(name="maskp", bufs=1))
                    ktp = p2.enter_context(tc.tile_pool(name="kt", bufs=1))
                    wop = p2.enter_context(tc.tile_pool(name="wo", bufs=1))
                    ptp = p2.enter_context(tc.tile_pool(name="pt", bufs=5))
                    rawp = p2.enter_context(tc.tile_pool(name="raw", bufs=4))
                    rrowp = p2.enter_context(tc.tile_pool(name="rrow", bufs=3))
                    tmpp = p2.enter_context(tc.tile_pool(name="tmp", bufs=3))
                    sps = p2.enter_context(
                        tc.tile_pool(name="sps", bufs=2, space="PSUM"))
                    spds = p2.enter_context(
                        tc.tile_pool(name="spds", bufs=1, space="PSUM"))
                    pvps = p2.enter_context(
                        tc.tile_pool(name="pvps", bufs=2, space="PSUM"))
                    bcps = p2.enter_context(
                        tc.tile_pool(name="bcps", bufs=1, space="PSUM"))

                    mask_sb = maskp.tile([128, MW], F32R)
                    nc.sync.dma_start(mask_sb[:], mask_d.ap())
                    kt_sb = []
                    for hp in range(HP):
                        k_ = ktp.tile([128, T], F32R, tag=f"kt{hp}",
                                      name=f"kt{hp}")
                        nc.sync.dma_start(
                            k_[:], kT_spill.ap()[hp * 128:(hp + 1) * 128, :])
                        kt_sb.append(k_)
                    wo_sb = []
                    for hp in range(HP):
                        w_ = wop.tile([128, C], F32R, tag=f"wo{hp}",
                                      name=f"wo{hp}")
                        nc.sync.dma_start(
                            w_[:], woT_d.ap()[hp * 128:(hp + 1) * 128, :])
                        wo_sb.append(w_)

                    for h in range(HPC):
                        hp, hl = h // 2, h % 2
                        psl = slice(hl * 64, hl * 64 + 64)
                        vsl = slice(h * VW, h * VW + 65)
                        for qsb in range(NQSB):
                            qbase = qsb * 512
                            n_full = 4 * qsb
                            pv = pvps.tile([128, 512], F32, tag="pv")
                            first = True
                            for kbp in range(n_full // 2):
                                kb0, kb1 = 2 * kbp, 2 * kbp + 1
                                sp_ = sps.tile([128, 1024], F32, tag="sp")
                                for u, kb in enumerate((kb0, kb1)):
                                    nc.tensor.matmul(
                                        sp_[:, u * 512:(u + 1) * 512],
                                        kt_sb[hp][psl, kb * 128:(kb + 1) * 128],
                                        qt_sb[hp][psl, qbase:qbase + 512],
                                        start=True, stop=True,
                                        skip_group_check=True)
                                pt = ptp.tile([128, MW], F32R, tag="pt")
                                nc.scalar.activation(pt[:, 0:1024], sp_[:],
                                                     EXP, scale=0.125)
                                for u, kb in enumerate((kb0, kb1)):
                                    nc.tensor.matmul(
                                        pv[0:65, :], v_sb[kb][:, vsl],
                                        pt[:, u * 512:(u + 1) * 512],
                                        start=first, stop=False,
                                        skip_group_check=True)
                                    first = False
                            # diagonal staircase: j0,j1 packed in a 2-bank
                            # psum tile, j2,j3 in a 1-bank tile; gapless
                            sp_a = sps.tile([128, 1024], F32, tag="sp")
                            sp_b = spds.tile([128, 512], F32, tag="spd")
                            diag_dst = (
                                (sp_a, 0), (sp_a, 512), (sp_b, 0), (sp_b, 256))
                            for j in range(4):
                                kb = n_full + j
                                n_ = 512 - QOFF[j]
                                dst, o_ = diag_dst[j]
                                nc.tensor.matmul(
                                    dst[:, o_:o_ + n_],
                                    kt_sb[hp][psl, kb * 128:(kb + 1) * 128],
                                    qt_sb[hp][psl,
                                              qbase + QOFF[j]:qbase + 512],
                                    start=True, stop=True,
                                    skip_group_check=True)
                            pt = ptp.tile([128, MW], F32R, tag="pt")
                            nc.scalar.activation(pt[:, 0:896], sp_a[:, 0:896],
                                                 EXP, scale=0.125)
                            nc.vector.tensor_mul(pt[:, 0:896], pt[:, 0:896],
                                                 mask_sb[:, 0:896])
                            nc.scalar.activation(pt[:, 896:MW], sp_b[:],
                                                 EXP, scale=0.125)
                            nc.vector.tensor_mul(pt[:, 896:MW], pt[:, 896:MW],
                                                 mask_sb[:, 896:MW])
                            for j in range(4):
                                kb = n_full + j
                                n_ = 512 - QOFF[j]
                                nc.tensor.matmul(
                                    pv[0:65, QOFF[j]:512], v_sb[kb][:, vsl],
                                    pt[:, POFF[j]:POFF[j] + n_],
                                    start=first, stop=(j == 3),
                                    skip_group_check=True)
                                first = False
                            # normalize: ctx = raw[0:64] / raw[64]
                            raw = rawp.tile([65, 512], F32)
                            nc.vector.tensor_copy(raw[:], pv[0:65, :])
                            rrow = rrowp.tile([65, 512], F32R)
                            with nc.allow_low_precision("softmax denom f32r"):
                                nc.vector.reciprocal(rrow[64:65, :],
                                                     raw[64:65, :])
                            bc = bcps.tile([64, 512], F32)
                            nc.tensor.matmul(bc[:], ones_r[64:65, :],
                                             rrow[64:65, :],
                                             start=True, stop=True,
                                             skip_group_check=True)
                            if hl == 0:
                                nc.vector.tensor_mul(
                                    ctx_sb[hp][0:64, qbase:qbase + 512],
                                    raw[0:64, :], bc[:])
                            else:
                                tmp = tmpp.tile([64, 512], F32R)
                                nc.vector.tensor_mul(tmp[:], raw[0:64, :],
                                                     bc[:])
                                nc.sync.dma_start(
                                    ctx_sb[hp][64:128, qbase:qbase + 512],
                                    tmp[:])

                    # -------------- phase 3: output projection --------------
                    with contextlib.ExitStack() as p3:
                        yp = p3.enter_context(tc.tile_pool(name="y", bufs=3))
                        for oi in range(8):
                            osl = slice(oi * 128, (oi + 1) * 128)
                            for tj in range(NQSB):
                                tsl = slice(tj * 512, (tj + 1) * 512)
                                ps_ = pvps.tile([128, 512], F32, tag="pv",
                                                name="yacc")
                                for hp in range(HP):
                                    nc.tensor.matmul(
                                        ps_[:], wo_sb[hp][:, osl],
                                        ctx_sb[hp][:, tsl],
                                        start=(hp == 0), stop=(hp == HP - 1),
                                        skip_group_check=True)
                                y_ = yp.tile([128, 512], F32)
                                nc.vector.tensor_scalar_add(
                                    y_[:], ps_[:], bias_sb[:, oi:oi + 1])
                                nc.sync.dma_start(yT_d.ap()[osl, tsl], y_[:])

        if iters == 1:
            emit()
        else:
            with tc.For_i(0, iters, 1):
                emit()
    nc.compile()
    return nc


def make_masks():
    """Merged staircase mask [128, MW]: psum col POFF[j] + (q - QOFF[j])
    holds causal keep-bit for key row k = 128*j + k_local vs query q."""
    m = np.zeros((128, MW), np.float32)
    k = np.arange(128)[:, None]
    for j in range(4):
        q = np.arange(QOFF[j], 512)[None, :]
        m[:, POFF[j]:POFF[j] + 512 - QOFF[j]] = (q >= 128 * j + k)
    return m


def shard_inputs(x, w_qkv, w_out, b_out):
    """Full inputs -> list of 8 per-core input dicts."""
    x = np.asarray(x, dtype=np.float32)
    w_qkv = np.asarray(w_qkv, dtype=np.float32)
    w_out = np.asarray(w_out, dtype=np.float32)
    b_out = np.asarray(b_out, dtype=np.float32)
    masks = make_masks()
    in_maps = []
    for c in range(N_CORES):
        b, hg = c // 2, c % 2
        h0 = hg * HPC
        csl = slice(h0 * D, (h0 + HPC) * D)
        im = {
            "xT": np.ascontiguousarray(x[b].T),
            "wqT": np.ascontiguousarray(w_qkv[0 * C:1 * C][csl].T),
            "wkT": np.ascontiguousarray(w_qkv[1 * C:2 * C][csl].T),
            "wvT": np.ascontiguousarray(w_qkv[2 * C:3 * C][csl].T),
            "woT": np.ascontiguousarray(w_out[:, csl].T),
            "bias": (np.ascontiguousarray(b_out.reshape(8, 128).T)
                     if hg == 0 else np.zeros((128, 8), np.float32)),
            "masks": masks,
        }
        in_maps.append(im)
    return in_maps


def gather_outputs(results):
    """8 per-core {'yT': [C,T]} -> full [B,T,C]."""
    y = np.empty((B, T, C), np.float32)
    for b in range(B):
        acc = results[2 * b]["yT"] + results[2 * b + 1]["yT"]
        y[b] = acc.T
    return y


def kernel(**inputs):
    from concourse.bass_utils import run_bass_kernel_spmd
    if "nc" not in _CACHE:
        _CACHE["nc"] = build_nc()
    nc = _CACHE["nc"]
    in_maps = shard_inputs(inputs["x"], inputs["w_qkv"],
                           inputs["w_out"], inputs["b_out"])
    res = run_bass_kernel_spmd(nc, in_maps, list(range(N_CORES)))
    return gather_outputs(res.results)



# revision 14
# speedup vs baseline: 1.7344x; 1.7344x over previous
"""Causal multi-head attention (B=4, T=2048, C=1024, H=16) on 8 TRN2 cores.

Sharding: batch (4) x head-group (2 groups of 8 heads) -> 8 shards, one per
core. Each core computes QKV projections for its 8 heads, causal attention,
and a Megatron row-parallel slice of the output projection; the host sums the
two head-group partial outputs per batch element.

v2 (vs v1): all matmul operands bf16 (f32 PSUM accumulation), K resident in
SBUF (no DRAM spill), transposed PV form (ctx[q,d] with the softmax
denominator accumulated through an appended ones column of V), per-partition
normalization on DVE, PE-transpose of normalized ctx into ctx^T for the
output projection, and fine-grained interleaving of projection / attention /
output-projection PE work so the Activation engine's exp stream stays
overlapped with matmuls instead of serializing behind them.

Self-contained: hardcodes shapes from the problem spec; no file reads.
"""
import sys
sys.path.insert(0, '/opt/trn_rl_repo')
import numpy as np
import ml_dtypes

BF = ml_dtypes.bfloat16

B, T, C = 4, 2048, 1024
H, D = 16, 64
N_CORES = 8
HPC = 8        # heads per core
HP = 4         # head pairs per core
KB = 16        # 128-row key tiles per sequence
NQSB = 4       # 512-column query superblocks
CI = 8         # 128-row contraction tiles over C
VW = 65        # V_aug stride per head (64 V cols + 1 ones col)

# Diagonal staircase inside one query superblock: key tile j (local) covers
# queries [QOFF[j], 512). Packed psum/mask layout: j0,j1 in tile A at cols
# 0/512; j2,j3 in tile B at cols 0/256. Mask columns: A then B.
QOFF = (0, 128, 256, 384)
DW = (512, 384, 256, 128)
AOFF = (0, 512)        # diag A packing (j0, j1) -> 896 cols
BOFF = (0, 256)        # diag B packing (j2, j3) -> 384 cols
MW = 1280              # mask width: 896 (A) + 384 (B)

_CACHE = {}


def build_nc(iters=1):
    import contextlib
    from collections import deque
    import concourse.tile as tile
    from concourse import bacc, mybir

    F32 = mybir.dt.float32
    BF16 = mybir.dt.bfloat16
    EXP = mybir.ActivationFunctionType.Exp

    nc = bacc.Bacc("TRN2", target_bir_lowering=False, debug=False)

    xT_d = nc.dram_tensor("xT", [C, T], BF16, kind="ExternalInput")
    wqT_d = nc.dram_tensor("wqT", [C, 512], BF16, kind="ExternalInput")
    wkT_d = nc.dram_tensor("wkT", [C, 512], BF16, kind="ExternalInput")
    wvT_d = nc.dram_tensor("wvT", [C, 512], BF16, kind="ExternalInput")
    woT_d = nc.dram_tensor("woT", [512, C], BF16, kind="ExternalInput")
    bias_d = nc.dram_tensor("bias", [128, 8], F32, kind="ExternalInput")
    mask_d = nc.dram_tensor("masks", [128, MW], BF16, kind="ExternalInput")
    ident_d = nc.dram_tensor("ident", [128, 128], BF16, kind="ExternalInput")
    yT_d = nc.dram_tensor("yT", [C, T], F32, kind="ExternalOutput")

    with tile.TileContext(nc) as tc:
        def emit():
            with contextlib.ExitStack() as es:
                const = es.enter_context(tc.tile_pool(name="const", bufs=1))
                bigp = es.enter_context(tc.tile_pool(name="big", bufs=1))
                ptp = es.enter_context(tc.tile_pool(name="ptp", bufs=24))
                stagep = es.enter_context(tc.tile_pool(name="stg", bufs=1))
                recipp = es.enter_context(tc.tile_pool(name="rcp", bufs=4))
                yp = es.enter_context(tc.tile_pool(name="y", bufs=3))
                sps = es.enter_context(
                    tc.tile_pool(name="sps", bufs=3, space="PSUM"))
                cps = es.enter_context(
                    tc.tile_pool(name="cps", bufs=2, space="PSUM"))

                ident_sb = const.tile([128, 128], BF16)
                bias_sb = const.tile([128, 8], F32)
                mask_sb = const.tile([128, MW], BF16)

                xt_sb, wv_sb, wq_sb, wk_sb = [], [], [], []
                for ci in range(CI):
                    t_ = bigp.tile([128, T], BF16, tag=f"xt{ci}")
                    nc.sync.dma_start(t_[:],
                                      xT_d.ap()[ci * 128:(ci + 1) * 128, :])
                    xt_sb.append(t_)
                for ci in range(CI):
                    t_ = bigp.tile([128, 512], BF16, tag=f"wv{ci}")
                    nc.sync.dma_start(t_[:],
                                      wvT_d.ap()[ci * 128:(ci + 1) * 128, :])
                    wv_sb.append(t_)
                for ci in range(CI):
                    t_ = bigp.tile([128, 512], BF16, tag=f"wq{ci}")
                    nc.sync.dma_start(t_[:],
                                      wqT_d.ap()[ci * 128:(ci + 1) * 128, :])
                    wq_sb.append(t_)
                for ci in range(CI):
                    t_ = bigp.tile([128, 512], BF16, tag=f"wk{ci}")
                    nc.sync.dma_start(t_[:],
                                      wkT_d.ap()[ci * 128:(ci + 1) * 128, :])
                    wk_sb.append(t_)
                nc.sync.dma_start(ident_sb[:], ident_d.ap())
                nc.sync.dma_start(bias_sb[:], bias_d.ap())
                nc.sync.dma_start(mask_sb[:], mask_d.ap())
                wo_sb = []
                for hp in range(HP):
                    t_ = bigp.tile([128, C], BF16, tag=f"wo{hp}")
                    nc.sync.dma_start(t_[:],
                                      woT_d.ap()[hp * 128:(hp + 1) * 128, :])
                    wo_sb.append(t_)

                qt_sb, kt_sb, ctx_sb, v_sb = [], [], [], []
                for hp in range(HP):
                    qt_sb.append(bigp.tile([128, T], BF16, tag=f"qt{hp}",
                                           name=f"qt{hp}"))
                    kt_sb.append(bigp.tile([128, T], BF16, tag=f"kt{hp}",
                                           name=f"kt{hp}"))
                    ctx_sb.append(bigp.tile([128, T], BF16, tag=f"ctx{hp}",
                                            name=f"ctx{hp}"))
                for kb in range(KB):
                    v_sb.append(bigp.tile([128, HPC * VW], BF16,
                                          tag=f"v{kb}", name=f"v{kb}"))
                stage_sb = [stagep.tile([128, 128], BF16, tag=f"st{q}",
                                        name=f"st{q}")
                            for q in range(KB)]

                # ---------------- PE work-group emitters ----------------
                def v_group(ti):
                    ps_ = sps.tile([128, 1024], F32, tag="sp", name="sp")
                    for ci in range(CI):
                        nc.tensor.matmul(
                            ps_[:, 0:512],
                            xt_sb[ci][:, ti * 128:(ti + 1) * 128],
                            wv_sb[ci][:],
                            start=(ci == 0), stop=(ci == CI - 1),
                            skip_group_check=True)
                    sv = v_sb[ti][:].rearrange("p (h w) -> p h w", w=VW)
                    pv = ps_[:, 0:512].rearrange("p (h w) -> p h w", w=64)
                    nc.vector.tensor_copy(sv[:, :, 0:64], pv)
                    nc.gpsimd.memset(sv[:, :, 64:65], 1.0)

                def qk_group(hp, tj, which):
                    w_sb = wq_sb if which == 'q' else wk_sb
                    dst = qt_sb[hp] if which == 'q' else kt_sb[hp]
                    fsl = slice(hp * 128, (hp + 1) * 128)
                    tsl = slice(tj * 512, (tj + 1) * 512)
                    ps_ = sps.tile([128, 1024], F32, tag="sp", name="sp")
                    for ci in range(CI):
                        nc.tensor.matmul(
                            ps_[:, 0:512], w_sb[ci][:, fsl],
                            xt_sb[ci][:, tsl],
                            start=(ci == 0), stop=(ci == CI - 1),
                            skip_group_check=True)
                    nc.vector.tensor_copy(dst[:, tsl], ps_[:, 0:512])

                def p3_group(tj, oi):
                    osl = slice(oi * 128, (oi + 1) * 128)
                    tsl = slice(tj * 512, (tj + 1) * 512)
                    ps_ = sps.tile([128, 1024], F32, tag="sp", name="sp")
                    for hp in range(HP):
                        nc.tensor.matmul(
                            ps_[:, 0:512], wo_sb[hp][:, osl],
                            ctx_sb[hp][:, tsl],
                            start=(hp == 0), stop=(hp == HP - 1),
                            skip_group_check=True)
                    y_ = yp.tile([128, 512], F32, name="yt")
                    nc.vector.tensor_scalar_add(y_[:], ps_[:, 0:512],
                                                bias_sb[:, oi:oi + 1])
                    nc.sync.dma_start(yT_d.ap()[osl, tsl], y_[:])

                # ---------------- interleave machinery ----------------
                filler = deque()
                state = {"acc": 0.0, "rate": 0.0}

                def point():
                    state["acc"] += state["rate"]
                    while state["acc"] >= 1.0 and filler:
                        state["acc"] -= 1.0
                        filler.popleft()()

                # ---------------- prologue ----------------
                # V for key tiles 0..7 and Q/K for head pair 0 up front;
                # the rest of V and later head pairs' Q/K interleave with
                # attention as PE filler between exp-gated S tiles.
                for ti in range(8):
                    v_group(ti)
                for tj in range(NQSB):
                    qk_group(0, tj, 'q')
                    qk_group(0, tj, 'k')

                # ---------------- attention ----------------
                for hp in range(HP):
                    if hp == 0:
                        for ti in range(8, KB):
                            filler.append(lambda ti=ti: v_group(ti))
                        for tj in range(NQSB):
                            filler.append(
                                lambda tj=tj: qk_group(1, tj, 'q'))
                            filler.append(
                                lambda tj=tj: qk_group(1, tj, 'k'))
                        state["rate"] = 0.55
                    elif hp < HP - 1:
                        for tj in range(NQSB):
                            filler.append(
                                lambda tj=tj, hp=hp: qk_group(hp + 1, tj, 'q'))
                            filler.append(
                                lambda tj=tj, hp=hp: qk_group(hp + 1, tj, 'k'))
                        state["rate"] = 0.15
                    else:
                        state["rate"] = 0.7

                    def s_pair(hp, hl, qsb, p2, ptloc):
                        prow = slice(hl * 64, hl * 64 + 64)
                        qbase = qsb * 512
                        ps_ = sps.tile([128, 1024], F32, tag="sp", name="sp")
                        for u in range(2):
                            kb = 2 * p2 + u
                            nc.tensor.matmul(
                                ps_[:, u * 512:(u + 1) * 512],
                                kt_sb[hp][prow, kb * 128:(kb + 1) * 128],
                                qt_sb[hp][prow, qbase:qbase + 512],
                                start=True, stop=True,
                                skip_group_check=True)
                        pt = ptp.tile([128, 1024], BF16, tag="pt", name="pt")
                        nc.scalar.activation(pt[:], ps_[:], EXP, scale=0.125)
                        for u in range(2):
                            ptloc[2 * p2 + u] = (pt, u * 512, 0)

                    def s_diag(hp, hl, qsb, half, ptloc):
                        prow = slice(hl * 64, hl * 64 + 64)
                        qbase = qsb * 512
                        nf = 4 * qsb
                        js = (0, 1) if half == 0 else (2, 3)
                        offs = AOFF if half == 0 else BOFF
                        w = 896 if half == 0 else 384
                        moff = 0 if half == 0 else 896
                        ps_ = sps.tile([128, 1024], F32, tag="sp", name="sp")
                        for u, j in enumerate(js):
                            kb = nf + j
                            nc.tensor.matmul(
                                ps_[:, offs[u]:offs[u] + DW[j]],
                                kt_sb[hp][prow, kb * 128:(kb + 1) * 128],
                                qt_sb[hp][prow,
                                          qbase + QOFF[j]:qbase + 512],
                                start=True, stop=True,
                                skip_group_check=True)
                        pt = ptp.tile([128, 1024], BF16, tag="pt", name="pt")
                        nc.scalar.activation(pt[:, 0:w], ps_[:, 0:w],
                                             EXP, scale=0.125)
                        nc.vector.tensor_mul(pt[:, 0:w], pt[:, 0:w],
                                             mask_sb[:, moff:moff + w])
                        for u, j in enumerate(js):
                            ptloc[nf + j] = (pt, offs[u], QOFF[j])

                    def pv_qt(hp, hl, qsb, qt, ptloc):
                        h = 2 * hp + hl
                        vsl = slice(h * VW, h * VW + VW)
                        qi = 4 * qsb + qt
                        qcol = qt * 128
                        ct = cps.tile([128, 512], F32, tag="cp", name="cp")
                        for kb in range(qi + 1):
                            pt, base, qs = ptloc[kb]
                            c0 = base + qcol - qs
                            nc.tensor.matmul(
                                ct[:, 0:VW],
                                pt[:, c0:c0 + 128],
                                v_sb[kb][:, vsl],
                                start=(kb == 0), stop=(kb == qi),
                                skip_group_check=True)
                        rc = recipp.tile([128, 1], F32, name="rc")
                        nc.vector.reciprocal(rc[:], ct[:, 64:65])
                        st = stage_sb[qsb * 4 + qt]
                        nc.vector.tensor_scalar_mul(
                            st[:, hl * 64:hl * 64 + 64], ct[:, 0:64], rc[:])
                        if hl == 1:
                            tp = cps.tile([128, 1024], BF16, tag="cp",
                                          name="tp")
                            nc.tensor.transpose(tp[:, 0:128], st[:],
                                                ident_sb[:])
                            nc.vector.tensor_copy(
                                ctx_sb[hp][:, qsb * 512 + qcol:
                                           qsb * 512 + qcol + 128],
                                tp[:, 0:128])

                    def head_units(hp, hl):
                        """Per-head emission units (closures), with the
                        one-superblock PV lag built in. Marker = qsb index
                        on the last PV chunk of that superblock."""
                        units = []
                        pend = []
                        for qsb in range(NQSB):
                            ptloc = {}
                            for p2 in range(2 * qsb):
                                units.append((lambda hp=hp, hl=hl, qsb=qsb,
                                              p2=p2, pl=ptloc:
                                              s_pair(hp, hl, qsb, p2, pl),
                                              None))
                                if pend:
                                    units.append(pend.pop(0))
                            for half in range(2):
                                units.append((lambda hp=hp, hl=hl, qsb=qsb,
                                              half=half, pl=ptloc:
                                              s_diag(hp, hl, qsb, half, pl),
                                              None))
                                if pend:
                                    units.append(pend.pop(0))
                            while pend:
                                units.append(pend.pop(0))
                            pend = [(lambda hp=hp, hl=hl, qsb=qsb, qt=qt,
                                     pl=ptloc: pv_qt(hp, hl, qsb, qt, pl),
                                     qsb if qt == NQSB - 1 else None)
                                    for qt in range(NQSB)]
                        while pend:
                            units.append(pend.pop(0))
                        return units

                    # interleave the pair's two heads unit-by-unit (h0 first
                    # at each step: h1's transpose reads h0's stage writes)
                    u0 = head_units(hp, 0)
                    u1 = head_units(hp, 1)
                    for i in range(max(len(u0), len(u1))):
                        for units in (u0, u1):
                            if i >= len(units):
                                continue
                            cl, marker = units[i]
                            cl()
                            point()
                            if (units is u1 and marker is not None
                                    and hp == HP - 1):
                                # ctx^T for this query superblock complete
                                # across all head pairs: emit its output
                                # projection as filler
                                for oi in range(8):
                                    filler.append(
                                        lambda oi=oi, tj=marker:
                                        p3_group(tj, oi))

                # drain whatever output-projection filler remains
                while filler:
                    filler.popleft()()

        if iters == 1:
            emit()
        else:
            with tc.For_i(0, iters, 1):
                emit()
    nc.compile()
    return nc


def make_masks():
    """Merged staircase mask [128, MW] (bf16 0/1): diag block j covers
    queries [QOFF[j], 512); keep iff q >= 128*j + k."""
    m = np.zeros((128, MW), np.float32)
    moffs = (0, 512, 896, 1152)
    k = np.arange(128)[:, None]
    for j in range(4):
        q = np.arange(QOFF[j], 512)[None, :]
        m[:, moffs[j]:moffs[j] + DW[j]] = (q >= 128 * j + k)
    return m.astype(BF)


def shard_inputs(x, w_qkv, w_out, b_out):
    """Full inputs -> list of 8 per-core input dicts (bf16 operands)."""
    x = np.asarray(x, dtype=np.float32)
    w_qkv = np.asarray(w_qkv, dtype=np.float32)
    w_out = np.asarray(w_out, dtype=np.float32)
    b_out = np.asarray(b_out, dtype=np.float32)
    masks = make_masks()
    ident = np.eye(128, dtype=BF)
    in_maps = []
    for c in range(N_CORES):
        b, hg = c // 2, c % 2
        h0 = hg * HPC
        csl = slice(h0 * D, (h0 + HPC) * D)
        im = {
            "xT": np.ascontiguousarray(x[b].T).astype(BF),
            "wqT": np.ascontiguousarray(w_qkv[0 * C:1 * C][csl].T).astype(BF),
            "wkT": np.ascontiguousarray(w_qkv[1 * C:2 * C][csl].T).astype(BF),
            "wvT": np.ascontiguousarray(w_qkv[2 * C:3 * C][csl].T).astype(BF),
            "woT": np.ascontiguousarray(w_out[:, csl].T).astype(BF),
            "bias": (np.ascontiguousarray(b_out.reshape(8, 128).T)
                     if hg == 0 else np.zeros((128, 8), np.float32)),
            "masks": masks,
            "ident": ident,
        }
        in_maps.append(im)
    return in_maps


def gather_outputs(results):
    """8 per-core {'yT': [C,T]} -> full [B,T,C]."""
    y = np.empty((B, T, C), np.float32)
    for b in range(B):
        acc = results[2 * b]["yT"] + results[2 * b + 1]["yT"]
        y[b] = acc.T
    return y


def kernel(**inputs):
    from concourse.bass_utils import run_bass_kernel_spmd
    if "nc" not in _CACHE:
        _CACHE["nc"] = build_nc()
    nc = _CACHE["nc"]
    in_maps = shard_inputs(inputs["x"], inputs["w_qkv"],
                           inputs["w_out"], inputs["b_out"])
    res = run_bass_kernel_spmd(nc, in_maps, list(range(N_CORES)))
    return gather_outputs(res.results)


# revision 16
# speedup vs baseline: 3.3077x; 1.9071x over previous
"""Causal multi-head attention (B=4, T=2048, C=1024, H=16) on 8 TRN2 cores.

Sharding: batch (4) x head-group (2 groups of 8 heads) -> 8 shards, one per
core. Each core computes QKV projections for its 8 heads, causal attention,
and a Megatron row-parallel slice of the output projection; the host sums the
two head-group partial outputs per batch element.

v2 (vs v1): all matmul operands bf16 (f32 PSUM accumulation), K resident in
SBUF (no DRAM spill), transposed PV form (ctx[q,d] with the softmax
denominator accumulated through an appended ones column of V), per-partition
normalization on DVE, PE-transpose of normalized ctx into ctx^T for the
output projection, and fine-grained interleaving of projection / attention /
output-projection PE work so the Activation engine's exp stream stays
overlapped with matmuls instead of serializing behind them.

Self-contained: hardcodes shapes from the problem spec; no file reads.
"""
import sys
sys.path.insert(0, '/opt/trn_rl_repo')
import numpy as np
import ml_dtypes

BF = ml_dtypes.bfloat16

B, T, C = 4, 2048, 1024
H, D = 16, 64
N_CORES = 8
HPC = 8        # heads per core
HP = 4         # head pairs per core
KB = 16        # 128-row key tiles per sequence
NQSB = 4       # 512-column query superblocks
CI = 8         # 128-row contraction tiles over C
VW = 65        # V_aug stride per head (64 V cols + 1 ones col)

# Diagonal staircase inside one query superblock: key tile j (local) covers
# queries [QOFF[j], 512). Packed psum/mask layout: j0,j1 in tile A at cols
# 0/512; j2,j3 in tile B at cols 0/256. Mask columns: A then B.
QOFF = (0, 128, 256, 384)
DW = (512, 384, 256, 128)
AOFF = (0, 512)        # diag A packing (j0, j1) -> 896 cols
BOFF = (0, 256)        # diag B packing (j2, j3) -> 384 cols
MW = 1280              # mask width: 896 (A) + 384 (B)

_CACHE = {}


def build_nc(iters=1):
    import contextlib
    from collections import deque
    import concourse.tile as tile
    from concourse import bacc, mybir

    F32 = mybir.dt.float32
    BF16 = mybir.dt.bfloat16
    EXP = mybir.ActivationFunctionType.Exp

    nc = bacc.Bacc("TRN2", target_bir_lowering=False, debug=False)

    xT_d = nc.dram_tensor("xT", [C, T], BF16, kind="ExternalInput")
    wqT_d = nc.dram_tensor("wqT", [C, 512], BF16, kind="ExternalInput")
    wkT_d = nc.dram_tensor("wkT", [C, 512], BF16, kind="ExternalInput")
    wvT_d = nc.dram_tensor("wvT", [C, 512], BF16, kind="ExternalInput")
    woT_d = nc.dram_tensor("woT", [512, C], BF16, kind="ExternalInput")
    bias_d = nc.dram_tensor("bias", [128, 8], F32, kind="ExternalInput")
    mask_d = nc.dram_tensor("masks", [128, MW], BF16, kind="ExternalInput")
    yT_d = nc.dram_tensor("yT", [C, T], F32, kind="ExternalOutput")

    with tile.TileContext(nc) as tc:
        def emit():
            with contextlib.ExitStack() as es:
                const = es.enter_context(tc.tile_pool(name="const", bufs=1))
                bigp = es.enter_context(tc.tile_pool(name="big", bufs=1))
                ptp = es.enter_context(tc.tile_pool(name="ptp", bufs=24))
                stagep = es.enter_context(tc.tile_pool(name="stg", bufs=1))
                recipp = es.enter_context(tc.tile_pool(name="rcp", bufs=4))
                yp = es.enter_context(tc.tile_pool(name="y", bufs=3))
                sps = es.enter_context(
                    tc.tile_pool(name="sps", bufs=3, space="PSUM"))
                cps = es.enter_context(
                    tc.tile_pool(name="cps", bufs=2, space="PSUM"))

                bias_sb = const.tile([128, 8], F32)
                mask_sb = const.tile([128, MW], BF16)

                xt_sb, wv_sb, wq_sb, wk_sb = [], [], [], []
                for ci in range(CI):
                    t_ = bigp.tile([128, T], BF16, tag=f"xt{ci}")
                    nc.sync.dma_start(t_[:],
                                      xT_d.ap()[ci * 128:(ci + 1) * 128, :])
                    xt_sb.append(t_)
                for ci in range(CI):
                    t_ = bigp.tile([128, 512], BF16, tag=f"wv{ci}")
                    nc.sync.dma_start(t_[:],
                                      wvT_d.ap()[ci * 128:(ci + 1) * 128, :])
                    wv_sb.append(t_)
                for ci in range(CI):
                    t_ = bigp.tile([128, 512], BF16, tag=f"wq{ci}")
                    nc.sync.dma_start(t_[:],
                                      wqT_d.ap()[ci * 128:(ci + 1) * 128, :])
                    wq_sb.append(t_)
                for ci in range(CI):
                    t_ = bigp.tile([128, 512], BF16, tag=f"wk{ci}")
                    nc.sync.dma_start(t_[:],
                                      wkT_d.ap()[ci * 128:(ci + 1) * 128, :])
                    wk_sb.append(t_)
                nc.sync.dma_start(bias_sb[:], bias_d.ap())
                nc.sync.dma_start(mask_sb[:], mask_d.ap())
                wo_sb = []
                for hp in range(HP):
                    t_ = bigp.tile([128, C], BF16, tag=f"wo{hp}")
                    nc.sync.dma_start(t_[:],
                                      woT_d.ap()[hp * 128:(hp + 1) * 128, :])
                    wo_sb.append(t_)

                qt_sb, kt_sb, ctx_sb, v_sb = [], [], [], []
                for hp in range(HP):
                    qt_sb.append(bigp.tile([128, T], BF16, tag=f"qt{hp}",
                                           name=f"qt{hp}"))
                    kt_sb.append(bigp.tile([128, T], BF16, tag=f"kt{hp}",
                                           name=f"kt{hp}"))
                    ctx_sb.append(bigp.tile([128, T], BF16, tag=f"ctx{hp}",
                                            name=f"ctx{hp}"))
                for kb in range(KB):
                    v_sb.append(bigp.tile([128, HPC * VW], BF16,
                                          tag=f"v{kb}", name=f"v{kb}"))
                stage_sb = [stagep.tile([128, 128], BF16, tag=f"st{q}",
                                        name=f"st{q}")
                            for q in range(KB)]

                # ---------------- PE work-group emitters ----------------
                def v_group(ti):
                    ps_ = sps.tile([128, 1024], F32, tag="sp", name="sp")
                    for ci in range(CI):
                        nc.tensor.matmul(
                            ps_[:, 0:512],
                            xt_sb[ci][:, ti * 128:(ti + 1) * 128],
                            wv_sb[ci][:],
                            start=(ci == 0), stop=(ci == CI - 1),
                            skip_group_check=True)
                    sv = v_sb[ti][:].rearrange("p (h w) -> p h w", w=VW)
                    pv = ps_[:, 0:512].rearrange("p (h w) -> p h w", w=64)
                    nc.vector.tensor_copy(sv[:, :, 0:64], pv)
                    nc.gpsimd.memset(sv[:, :, 64:65], 1.0)

                def qk_group(hp, tj, which):
                    w_sb = wq_sb if which == 'q' else wk_sb
                    dst = qt_sb[hp] if which == 'q' else kt_sb[hp]
                    fsl = slice(hp * 128, (hp + 1) * 128)
                    tsl = slice(tj * 512, (tj + 1) * 512)
                    ps_ = sps.tile([128, 1024], F32, tag="sp", name="sp")
                    for ci in range(CI):
                        nc.tensor.matmul(
                            ps_[:, 0:512], w_sb[ci][:, fsl],
                            xt_sb[ci][:, tsl],
                            start=(ci == 0), stop=(ci == CI - 1),
                            skip_group_check=True)
                    nc.vector.tensor_copy(dst[:, tsl], ps_[:, 0:512])

                def p3_group(tj, oi):
                    osl = slice(oi * 128, (oi + 1) * 128)
                    tsl = slice(tj * 512, (tj + 1) * 512)
                    ps_ = sps.tile([128, 1024], F32, tag="sp", name="sp")
                    for hp in range(HP):
                        nc.tensor.matmul(
                            ps_[:, 0:512], wo_sb[hp][:, osl],
                            ctx_sb[hp][:, tsl],
                            start=(hp == 0), stop=(hp == HP - 1),
                            skip_group_check=True)
                    y_ = yp.tile([128, 512], F32, name="yt")
                    nc.vector.tensor_scalar_add(y_[:], ps_[:, 0:512],
                                                bias_sb[:, oi:oi + 1])
                    nc.sync.dma_start(yT_d.ap()[osl, tsl], y_[:])

                # ---------------- interleave machinery ----------------
                filler = deque()
                state = {"acc": 0.0, "rate": 0.0}

                def point():
                    state["acc"] += state["rate"]
                    while state["acc"] >= 1.0 and filler:
                        state["acc"] -= 1.0
                        filler.popleft()()

                # ---------------- prologue ----------------
                # V for key tiles 0..7 and Q/K for head pair 0 up front;
                # the rest of V and later head pairs' Q/K interleave with
                # attention as PE filler between exp-gated S tiles.
                for ti in range(8):
                    v_group(ti)
                for tj in range(NQSB):
                    qk_group(0, tj, 'q')
                    qk_group(0, tj, 'k')

                # ---------------- attention ----------------
                for hp in range(HP):
                    if hp == 0:
                        for ti in range(8, KB):
                            filler.append(lambda ti=ti: v_group(ti))
                        for tj in range(NQSB):
                            filler.append(
                                lambda tj=tj: qk_group(1, tj, 'q'))
                            filler.append(
                                lambda tj=tj: qk_group(1, tj, 'k'))
                        state["rate"] = 0.55
                    elif hp < HP - 1:
                        for tj in range(NQSB):
                            filler.append(
                                lambda tj=tj, hp=hp: qk_group(hp + 1, tj, 'q'))
                            filler.append(
                                lambda tj=tj, hp=hp: qk_group(hp + 1, tj, 'k'))
                        state["rate"] = 0.15
                    else:
                        state["rate"] = 0.7

                    def s_pair(hp, hl, qsb, p2, ptloc):
                        prow = slice(hl * 64, hl * 64 + 64)
                        qbase = qsb * 512
                        ps_ = sps.tile([128, 1024], F32, tag="sp", name="sp")
                        for u in range(2):
                            kb = 2 * p2 + u
                            nc.tensor.matmul(
                                ps_[:, u * 512:(u + 1) * 512],
                                kt_sb[hp][prow, kb * 128:(kb + 1) * 128],
                                qt_sb[hp][prow, qbase:qbase + 512],
                                start=True, stop=True,
                                skip_group_check=True)
                        pt = ptp.tile([128, 1024], BF16, tag="pt", name="pt")
                        nc.scalar.activation(pt[:], ps_[:], EXP, scale=0.125)
                        for u in range(2):
                            ptloc[2 * p2 + u] = (pt, u * 512, 0)

                    def s_diag(hp, hl, qsb, half, ptloc):
                        prow = slice(hl * 64, hl * 64 + 64)
                        qbase = qsb * 512
                        nf = 4 * qsb
                        js = (0, 1) if half == 0 else (2, 3)
                        offs = AOFF if half == 0 else BOFF
                        w = 896 if half == 0 else 384
                        moff = 0 if half == 0 else 896
                        ps_ = sps.tile([128, 1024], F32, tag="sp", name="sp")
                        for u, j in enumerate(js):
                            kb = nf + j
                            nc.tensor.matmul(
                                ps_[:, offs[u]:offs[u] + DW[j]],
                                kt_sb[hp][prow, kb * 128:(kb + 1) * 128],
                                qt_sb[hp][prow,
                                          qbase + QOFF[j]:qbase + 512],
                                start=True, stop=True,
                                skip_group_check=True)
                        pt = ptp.tile([128, 1024], BF16, tag="pt", name="pt")
                        nc.scalar.activation(pt[:, 0:w], ps_[:, 0:w],
                                             EXP, scale=0.125)
                        nc.vector.tensor_mul(pt[:, 0:w], pt[:, 0:w],
                                             mask_sb[:, moff:moff + w])
                        for u, j in enumerate(js):
                            ptloc[nf + j] = (pt, offs[u], QOFF[j])

                    def pv_qt(hp, hl, qsb, qt, ptloc):
                        h = 2 * hp + hl
                        vsl = slice(h * VW, h * VW + VW)
                        qi = 4 * qsb + qt
                        qcol = qt * 128
                        ct = cps.tile([128, 512], F32, tag="cp", name="cp")
                        for kb in range(qi + 1):
                            pt, base, qs = ptloc[kb]
                            c0 = base + qcol - qs
                            nc.tensor.matmul(
                                ct[:, 0:VW],
                                pt[:, c0:c0 + 128],
                                v_sb[kb][:, vsl],
                                start=(kb == 0), stop=(kb == qi),
                                skip_group_check=True)
                        rc = recipp.tile([128, 1], F32, name="rc")
                        nc.vector.reciprocal(rc[:], ct[:, 64:65])
                        st = stage_sb[qsb * 4 + qt]
                        nc.vector.tensor_scalar_mul(
                            st[:, hl * 64:hl * 64 + 64], ct[:, 0:64], rc[:])
                        if hl == 1:
                            # ctx^T via the DMA xbar transpose unit: frees
                            # PE (no identity matmul) and DVE (no psum copy)
                            nc.sync.dma_start_transpose(
                                ctx_sb[hp][:, qsb * 512 + qcol:
                                           qsb * 512 + qcol + 128],
                                st[:])

                    def head_units(hp, hl):
                        """Per-head emission units (closures), with the
                        one-superblock PV lag built in. Marker = qsb index
                        on the last PV chunk of that superblock."""
                        units = []
                        pend = []
                        for qsb in range(NQSB):
                            ptloc = {}
                            for p2 in range(2 * qsb):
                                units.append((lambda hp=hp, hl=hl, qsb=qsb,
                                              p2=p2, pl=ptloc:
                                              s_pair(hp, hl, qsb, p2, pl),
                                              None))
                                if pend:
                                    units.append(pend.pop(0))
                            for half in range(2):
                                units.append((lambda hp=hp, hl=hl, qsb=qsb,
                                              half=half, pl=ptloc:
                                              s_diag(hp, hl, qsb, half, pl),
                                              None))
                                if pend:
                                    units.append(pend.pop(0))
                            while pend:
                                units.append(pend.pop(0))
                            pend = [(lambda hp=hp, hl=hl, qsb=qsb, qt=qt,
                                     pl=ptloc: pv_qt(hp, hl, qsb, qt, pl),
                                     qsb if qt == NQSB - 1 else None)
                                    for qt in range(NQSB)]
                        while pend:
                            units.append(pend.pop(0))
                        return units

                    # interleave the pair's two heads unit-by-unit (h0 first
                    # at each step: h1's transpose reads h0's stage writes)
                    u0 = head_units(hp, 0)
                    u1 = head_units(hp, 1)
                    for i in range(max(len(u0), len(u1))):
                        for units in (u0, u1):
                            if i >= len(units):
                                continue
                            cl, marker = units[i]
                            cl()
                            point()
                            if (units is u1 and marker is not None
                                    and hp == HP - 1):
                                # ctx^T for this query superblock complete
                                # across all head pairs: emit its output
                                # projection as filler
                                for oi in range(8):
                                    filler.append(
                                        lambda oi=oi, tj=marker:
                                        p3_group(tj, oi))

                # drain whatever output-projection filler remains
                while filler:
                    filler.popleft()()

        if iters == 1:
            emit()
        else:
            with tc.For_i(0, iters, 1):
                emit()
    nc.compile()
    return nc


def make_masks():
    """Merged staircase mask [128, MW] (bf16 0/1): diag block j covers
    queries [QOFF[j], 512); keep iff q >= 128*j + k."""
    m = np.zeros((128, MW), np.float32)
    moffs = (0, 512, 896, 1152)
    k = np.arange(128)[:, None]
    for j in range(4):
        q = np.arange(QOFF[j], 512)[None, :]
        m[:, moffs[j]:moffs[j] + DW[j]] = (q >= 128 * j + k)
    return m.astype(BF)


def shard_inputs(x, w_qkv, w_out, b_out):
    """Full inputs -> list of 8 per-core input dicts (bf16 operands)."""
    x = np.asarray(x, dtype=np.float32)
    w_qkv = np.asarray(w_qkv, dtype=np.float32)
    w_out = np.asarray(w_out, dtype=np.float32)
    b_out = np.asarray(b_out, dtype=np.float32)
    masks = make_masks()
    in_maps = []
    for c in range(N_CORES):
        b, hg = c // 2, c % 2
        h0 = hg * HPC
        csl = slice(h0 * D, (h0 + HPC) * D)
        im = {
            "xT": np.ascontiguousarray(x[b].T).astype(BF),
            "wqT": np.ascontiguousarray(w_qkv[0 * C:1 * C][csl].T).astype(BF),
            "wkT": np.ascontiguousarray(w_qkv[1 * C:2 * C][csl].T).astype(BF),
            "wvT": np.ascontiguousarray(w_qkv[2 * C:3 * C][csl].T).astype(BF),
            "woT": np.ascontiguousarray(w_out[:, csl].T).astype(BF),
            "bias": (np.ascontiguousarray(b_out.reshape(8, 128).T)
                     if hg == 0 else np.zeros((128, 8), np.float32)),
            "masks": masks,
        }
        in_maps.append(im)
    return in_maps


def gather_outputs(results):
    """8 per-core {'yT': [C,T]} -> full [B,T,C]."""
    y = np.empty((B, T, C), np.float32)
    for b in range(B):
        acc = results[2 * b]["yT"] + results[2 * b + 1]["yT"]
        y[b] = acc.T
    return y


def kernel(**inputs):
    from concourse.bass_utils import run_bass_kernel_spmd
    if "nc" not in _CACHE:
        _CACHE["nc"] = build_nc()
    nc = _CACHE["nc"]
    in_maps = shard_inputs(inputs["x"], inputs["w_qkv"],
                           inputs["w_out"], inputs["b_out"])
    res = run_bass_kernel_spmd(nc, in_maps, list(range(N_CORES)))
    return gather_outputs(res.results)
